# revision 1
# baseline (speedup 1.0000x reference)
"""4-D average pool (kernel=2, stride=2) over [2,16,32,32,32,32] f32, on 8 NeuronCores.

Strategy: data-parallel over the 32 (b,c) slices -> 4 slices per core; the
per-core input is a contiguous [4096, 1024] f32 block (rows = (slice,d1,d2),
cols = (d3,d4)).

Fully-contiguous loads on the SP HWDGE ring, 2 MiB for the bulk and tapering
to 512 KiB at the end (rows stay the partition dim - strided gathers measure
~2x slower on HBM under 8-core load; small final loads shorten the tail
chain).  The whole 16 MiB shard is SBUF-resident, so load DMAs carry no
waits and stream back-to-back at ~400 GB/s.  Compute runs in 256-row blocks:
  - two DVE adds pool the free dim (d4 pairs, then d3 pairs) -> [128, 512]
  - ONE fp32 matmul with a constant [128, 32] pooling matrix (stationary
    weights, 32-column LDWEIGHTS is ~free) pools the (d1,d2) partition
    pairs for both 128-row chunks at once -> PSUM [32, 512]
  - ScalarE copies PSUM->SBUF; the store DMA (ACT HWDGE ring) scatters the
    32-row chunks to their output rows.
The 1/16 average scale is folded into the pooling matrix.
"""

import sys

import numpy as np

if "/opt/trn_rl_repo" not in sys.path:
    sys.path.insert(0, "/opt/trn_rl_repo")

import concourse.bacc as bacc
import concourse.bass as bass
import concourse.tile as tile
from concourse import mybir
from concourse.bass_utils import run_bass_kernel_spmd

N_CORES = 8
SLICES_PER_CORE = 4  # 32 (b,c) slices / 8 cores
ROWS = SLICES_PER_CORE * 1024  # 4096
# DMA schedule (start_row, n_rows): big 2 MiB loads for the bulk (best
# stream rate), tapering to 512 KiB at the end to shorten the tail chain.
# Compute runs in uniform 256-row (or final 128-row) blocks within each load.
LOADS = [(r, 512) for r in range(0, 3584, 512)] + [(3584, 256), (3840, 128), (3968, 128)]
F32 = mybir.dt.float32


def _build_pm() -> np.ndarray:
    # B[r, j] = 1/16 iff chunk row r = 32*d1l + d2 pools into chunk output
    # row j = 16*(d1l//2) + d2//2   (d1l in [0,4), d2 in [0,32))
    b = np.zeros((128, 32), np.float32)
    for d1l in range(4):
        for d2 in range(32):
            b[32 * d1l + d2, 16 * (d1l // 2) + d2 // 2] = 1.0 / 16.0
    return b


def build_nc() -> bass.Bass:
    # Bacc (not raw Bass): its compile() splits multi-sem sync waits into
    # event-semaphore instructions (TRN2 allows one wait per instruction).
    nc = bacc.Bacc()
    x = nc.dram_tensor("x", [ROWS, 1024], F32, kind="ExternalInput")
    pm = nc.dram_tensor("pm", [128, 32], F32, kind="ExternalInput")
    y = nc.dram_tensor("y", [ROWS // 4, 256], F32, kind="ExternalOutput")

    n2m = sum(1 for _, n in LOADS if n == 512)
    n1m = sum(1 for _, n in LOADS if n == 256)
    nhm = sum(1 for _, n in LOADS if n == 128)

    with tile.TileContext(nc) as tc:
        with (
            tc.tile_pool(name="pmp", bufs=1) as pmp,
            # one pool per load size, bufs = count -> no slot reuse; the
            # whole 16 MiB input is SBUF-resident
            tc.tile_pool(name="in2m", bufs=max(n2m, 1)) as in2m,
            tc.tile_pool(name="in1m", bufs=max(n1m, 1)) as in1m,
            tc.tile_pool(name="inhm", bufs=max(nhm, 1)) as inhm,
            tc.tile_pool(name="m1p", bufs=8) as m1p,
            tc.tile_pool(name="m2p", bufs=8) as m2p,
            tc.tile_pool(name="psp", bufs=8, space=bass.MemorySpace.PSUM) as psp,
            tc.tile_pool(name="obp", bufs=8) as obp,
        ):
            pm_t = pmp.tile([128, 32], F32)
            pools = {512: in2m, 256: in1m, 128: inhm}

            def emit_block(tv, nq, d3, orow, oc0):
                # pool d4 pairs: [128, q, d3, 16o4, 2e4] -> [128, 16*d3*q]
                o3 = d3 // 2
                v = tv.rearrange(
                    "p (q d3 o4 e4) -> p q d3 o4 e4", q=nq, d3=d3, o4=16
                )
                m1 = m1p.tile([128, 16 * d3 * nq], F32, tag="m1")
                m1v = m1[:].rearrange("p (q d3 o4) -> p q d3 o4", q=nq, d3=d3)
                nc.vector.tensor_add(m1v, v[:, :, :, :, 0], v[:, :, :, :, 1])

                # pool d3 pairs -> [128, 16*o3*q]
                w = m1[:].rearrange(
                    "p (q o3 e3 o4) -> p q o3 e3 o4", q=nq, o3=o3, o4=16
                )
                m2 = m2p.tile([128, 16 * o3 * nq], F32, tag="m2")
                m2v = m2[:].rearrange("p (q o3 o4) -> p q o3 o4", q=nq, o3=o3)
                nc.vector.tensor_add(m2v, w[:, :, :, 0, :], w[:, :, :, 1, :])

                # pool (d1,d2) partition pairs in one matmul
                ps = psp.tile([32, 16 * o3 * nq], F32, tag="ps")
                nc.tensor.matmul(ps[:], pm_t[:], m2[:], start=True, stop=True)

                ob = obp.tile([32, 16 * o3 * nq], F32, tag="ob")
                nc.scalar.copy(ob[:], ps[:])

                # chunk q lands at output rows orow + 32q, cols [oc0, oc0+16*o3)
                dst = y[orow : orow + 32 * nq, oc0 : oc0 + 16 * o3].rearrange(
                    "(q j) c -> j q c", j=32
                )
                nc.scalar.dma_start(dst, ob[:].rearrange("j (q c) -> j q c", q=nq))

            for li, (row, nrows) in enumerate(LOADS):
                # contiguous load: nrows/128 chunks of 128 input rows side by
                # side in the free dim.  All in-DMAs stay on the SP ring: the
                # ACT ring's triggers sit behind copies that wait on matmuls
                # (head-of-line blocking stalls the stream).
                nqt = nrows // 128
                t = pools[nrows].tile([128, 1024 * nqt], F32, tag="t")
                src = x[row : row + nrows, :].rearrange("(q p) c -> p q c", p=128)
                nc.sync.dma_start(t[:].rearrange("p (q c) -> p q c", q=nqt), src)
                if li == 0:
                    # pm load after the first bulk DMA: it is only needed by
                    # the first matmul (~14 us in), not on the critical path
                    nc.sync.dma_start(pm_t[:], pm[:])

                # compute blocks of <=2 chunks within the load
                for qi in range(0, nqt, 2):
                    nq = min(2, nqt - qi)
                    tv = t[:, 1024 * qi : 1024 * (qi + nq)]
                    emit_block(tv, nq, 32, row // 4 + 32 * qi, 0)

    nc.compile()
    return nc


_NC_CACHE: bass.Bass | None = None


def kernel(nd_tensor: np.ndarray, _trace: bool = False):
    global _NC_CACHE
    x = np.ascontiguousarray(np.asarray(nd_tensor, dtype=np.float32)).reshape(
        32, 1024, 1024
    )
    pm = _build_pm()
    if _NC_CACHE is None:
        _NC_CACHE = build_nc()
    nc = _NC_CACHE

    in_maps = [
        {
            "x": np.ascontiguousarray(
                x[SLICES_PER_CORE * i : SLICES_PER_CORE * (i + 1)]
            ).reshape(ROWS, 1024),
            "pm": pm,
        }
        for i in range(N_CORES)
    ]
    res = run_bass_kernel_spmd(
        nc, in_maps, core_ids=list(range(N_CORES)), trace=_trace
    )
    out = np.stack([res.results[i]["y"] for i in range(N_CORES)])  # [8,1024,256]
    out = out.reshape(2, 16, 16, 16, 16, 16).astype(np.float32)
    if _trace:
        kernel.last_results = res
    return out



# revision 2
# speedup vs baseline: 1.6901x; 1.6901x over previous
"""4-D average pool (kernel=2, stride=2) over [2,16,32,32,32,32] f32, on 8 NeuronCores.

Strategy: data-parallel over the 32 (b,c) slices -> 4 slices per core.  The
host casts the input to bf16 during sharding (tolerance is 2e-2; bf16 keeps
worst-case error ~1e-2 with margin), halving the HBM stream to 8 MiB/core.

Per-core layout: [4096, 1024] bf16, rows = (slice,d1,d2), cols = (d3,d4).
8 loads of 512 rows each (1 MiB), p-major: partition p holds 4 consecutive
rows (one d1, four d2) = 8 KiB contiguous HBM per partition -> max-size
descriptors, the load stream runs back-to-back at HBM rate.

Compute per load tile [128, (4 d2, 32 d3, 32 d4)] (all bf16, DVE 2x rate):
  - add A pools d3 pairs (contiguous 64 B runs)        -> [128, (4,16,32)]
  - add B pools d2 pairs (contiguous 1 KiB runs)       -> [128, (2,16,32)]
  - add C pools d4 pairs (stride-2, small)             -> [128, (2,16,16)]
  - one single-pass bf16 matmul with a [128,64] pooling matrix sums the
    d1 partition pairs and applies the 1/16 scale      -> PSUM [64,512] f32
  - ScalarE copies PSUM->SBUF f32; store DMA writes y[:, 512k:512k+512]
    (2 KiB contiguous per partition row).
Output y is [64, 4096] f32 per core; the host decodes the (q,k,f) layout
back to (B,C,16,16,16,16).
"""

import sys

import ml_dtypes
import numpy as np

if "/opt/trn_rl_repo" not in sys.path:
    sys.path.insert(0, "/opt/trn_rl_repo")

import concourse.bacc as bacc
import concourse.bass as bass
import concourse.tile as tile
from concourse import mybir
from concourse.bass_utils import run_bass_kernel_spmd

N_CORES = 8
SLICES_PER_CORE = 4  # 32 (b,c) slices / 8 cores
ROWS = SLICES_PER_CORE * 1024  # 4096
N_LOADS = 8
LROWS = ROWS // N_LOADS  # 512 rows = 1 MiB bf16 per load
BF16 = mybir.dt.bfloat16
F32 = mybir.dt.float32


def _build_pm() -> np.ndarray:
    # pm[p, q] = 1/16 for q = 8*(p//16) + p%8: partitions p and p+8 hold the
    # (d1, d1+1) pair for the same d2 block; 1/16 folds the average scale.
    b = np.zeros((128, 64), np.float32)
    for p in range(128):
        b[p, 8 * (p // 16) + p % 8] = 1.0 / 16.0
    return b.astype(ml_dtypes.bfloat16)


def build_nc() -> bass.Bass:
    nc = bacc.Bacc()
    x = nc.dram_tensor("x", [ROWS, 1024], BF16, kind="ExternalInput")
    pm = nc.dram_tensor("pm", [128, 64], BF16, kind="ExternalInput")
    y = nc.dram_tensor("y", [64, 512 * N_LOADS], F32, kind="ExternalOutput")

    with tile.TileContext(nc) as tc:
        with (
            tc.tile_pool(name="pmp", bufs=1) as pmp,
            # whole 8 MiB shard SBUF-resident: no slot reuse, loads carry no
            # waits and stream back-to-back
            tc.tile_pool(name="inp", bufs=N_LOADS) as inp,
            tc.tile_pool(name="m1p", bufs=3) as m1p,
            tc.tile_pool(name="m2p", bufs=3) as m2p,
            tc.tile_pool(name="m3p", bufs=3) as m3p,
            tc.tile_pool(name="psp", bufs=8, space=bass.MemorySpace.PSUM) as psp,
            tc.tile_pool(name="obp", bufs=4) as obp,
        ):
            pm_t = pmp.tile([128, 64], BF16)

            for k in range(N_LOADS):
                # p-major load: partition p <- rows [4p, 4p+4) of this chunk,
                # an 8 KiB contiguous HBM run per partition.
                t = inp.tile([128, 4096], BF16, tag="t")
                src = x[LROWS * k : LROWS * (k + 1), :].rearrange(
                    "(p r) c -> p r c", p=128
                )
                nc.sync.dma_start(t[:].rearrange("p (r c) -> p r c", r=4), src)
                if k == 0:
                    # pm only needed by the first matmul, ~3 us in
                    nc.sync.dma_start(pm_t[:], pm[:])

                # A: pool d3 pairs (contiguous runs of 32 elems)
                v = t[:].rearrange(
                    "p (r o3 e3 d4) -> p r o3 e3 d4", r=4, o3=16, d4=32
                )
                m1 = m1p.tile([128, 2048], BF16, tag="m1")
                m1v = m1[:].rearrange("p (r o3 d4) -> p r o3 d4", r=4, o3=16)
                nc.vector.tensor_add(m1v, v[:, :, :, 0, :], v[:, :, :, 1, :])

                # B: pool d2 pairs (contiguous runs of 512 elems)
                w = m1[:].rearrange("p (ro re f) -> p ro re f", ro=2, re=2)
                m2 = m2p.tile([128, 1024], BF16, tag="m2")
                m2v = m2[:].rearrange("p (ro f) -> p ro f", ro=2)
                nc.vector.tensor_add(m2v, w[:, :, 0, :], w[:, :, 1, :])

                # C: pool d4 pairs (stride-2, only 512 outputs per partition)
                u = m2[:].rearrange(
                    "p (ro o3 o4 e4) -> p ro o3 o4 e4", ro=2, o3=16, o4=16
                )
                m3 = m3p.tile([128, 512], BF16, tag="m3")
                m3v = m3[:].rearrange("p (ro o3 o4) -> p ro o3 o4", ro=2, o3=16)
                nc.vector.tensor_add(m3v, u[:, :, :, :, 0], u[:, :, :, :, 1])

                # d1 partition pairs + 1/16 scale in one bf16 matmul
                ps = psp.tile([64, 512], F32, tag="ps")
                nc.tensor.matmul(ps[:], pm_t[:], m3[:], start=True, stop=True)

                ob = obp.tile([64, 512], F32, tag="ob")
                nc.scalar.copy(ob[:], ps[:])
                nc.scalar.dma_start(y[:, 512 * k : 512 * (k + 1)], ob[:])

    nc.compile()
    return nc


_NC_CACHE: bass.Bass | None = None


def kernel(nd_tensor: np.ndarray, _trace: bool = False):
    global _NC_CACHE
    x = np.ascontiguousarray(np.asarray(nd_tensor, dtype=np.float32)).reshape(
        32, 1024, 1024
    )
    xb = x.astype(ml_dtypes.bfloat16)  # round-to-nearest-even
    pm = _build_pm()
    if _NC_CACHE is None:
        _NC_CACHE = build_nc()
    nc = _NC_CACHE

    in_maps = [
        {
            "x": np.ascontiguousarray(
                xb[SLICES_PER_CORE * i : SLICES_PER_CORE * (i + 1)]
            ).reshape(ROWS, 1024),
            "pm": pm,
        }
        for i in range(N_CORES)
    ]
    res = run_bass_kernel_spmd(
        nc, in_maps, core_ids=list(range(N_CORES)), trace=_trace
    )
    # y[q, 512k + f]: q = (o1l' 8, d2blk 8); k = (s_local 4, khalf 2);
    # f = (o2l 2, o3 16, o4 16).  o1 = 8*khalf + o1l', o2 = 2*d2blk + o2l.
    outs = []
    for i in range(N_CORES):
        yc = res.results[i]["y"].reshape(8, 8, 4, 2, 2, 16, 16)
        yc = yc.transpose(2, 3, 0, 1, 4, 5, 6).reshape(4, 16, 16, 16, 16)
        outs.append(yc)
    out = np.concatenate(outs, axis=0).reshape(2, 16, 16, 16, 16, 16)
    out = np.ascontiguousarray(out).astype(np.float32)
    if _trace:
        kernel.last_results = res
    return out
